# revision 27
# baseline (speedup 1.0000x reference)
"""PatchCore anomaly score kernel for 8 trn2 NeuronCores.

score = sqrt(max_n min_m ||patches[n] - memory_bank[m]||^2)

Device (per core, memory_bank sharded 4096 rows/core):
  acc[p, n] = max_mt (2*patches[n].bank[mt*128+p] + C - m_sq[mt*128+p])
Host:
  r[n] = max_c max_p acc_c[p, n]; min_d2[n] = p_sq[n] + C - r[n]
  score = sqrt(max_n min_d2), refined exactly for near-max candidates.

Pipeline: per m-step, 2 n-quads ([128,2048] PSUM, 4 banks each). Measured
isolated unit costs (512 cols): PE 2xDR matmul 181ns, ACT quad evac
~470ns, DVE quad merge ~290ns, DVE quad STT (bias+max straight from PSUM)
~570ns. Alternating quad mechanisms 3xACT : 1xSTT per 2 m-steps splits the
PSUM-drain between ACT and DVE under the 8-bank PSUM cap. Measured
~315.6us/iter (R-slope) vs 316us for the g1 ACT-only baseline; all
PSUM-drain structures pace ~600ns/unit in-kernel (sem/dispatch overhead
on top of engine rates - see memory trn2-psum-evac-rates).
"""

import sys

import numpy as np

try:
    import concourse.bass as bass
except ImportError:
    sys.path.insert(0, "/opt/trn_rl_repo")
    import concourse.bass as bass

import concourse.bacc as bacc
import concourse.tile as tile
from concourse import mybir
from concourse.bass_utils import run_bass_kernel_spmd

import ml_dtypes

N = 8192          # patches
M_TOTAL = 32768   # memory bank rows
D = 512           # feature dim
N_CORES = 8
M = M_TOTAL // N_CORES   # 4096 bank rows per core

KP = 4            # k-chunks of 128 over D
NT = N // 512     # 16 n-tiles of 512 patches
MT = M // 128     # 32 m-tiles of 128 bank rows
NG = 2            # n-groups (outer loop)
GT = NT // NG     # 8 n-tiles per group = 2 quads per m-step


def _build_nc(repeat=1, mode="full", mmw=512):
    # mmw: matmul output width; must stay 512 (single-bank writes) — the
    # compiler rejects matmul outputs that span PSUM banks
    nc = bacc.Bacc(None, target_bir_lowering=False)
    f32 = mybir.dt.float32
    bf16 = mybir.dt.bfloat16
    fp8 = mybir.dt.float8e4

    # dim1 index ci = d//128; slice [:, 2c:2c+2, :] = K rows [c*256,(c+1)*256)
    at_d = nc.dram_tensor("at", [128, KP, N], fp8, kind="ExternalInput")
    bt_d = nc.dram_tensor("bt", [128, KP, M], fp8, kind="ExternalInput")
    msq_d = nc.dram_tensor("msq", [128, MT], f32, kind="ExternalInput")
    # out[p, n] = max over m-tiles of 2*a.b + (C - m_sq); host maxes over p
    out_d = nc.dram_tensor("out", [128, N], bf16, kind="ExternalOutput")

    with tile.TileContext(nc) as tc:
        with (
            tc.tile_pool(name="at", bufs=1) as at_pool,
            tc.tile_pool(name="bt", bufs=1) as bt_pool,
            tc.tile_pool(name="msq", bufs=1) as msq_pool,
            tc.tile_pool(name="acc", bufs=1) as acc_pool,
            tc.tile_pool(name="ev", bufs=4) as ev_pool,
            tc.tile_pool(name="psum", bufs=2, space="PSUM") as psum_pool,
        ):
            msq_t = msq_pool.tile([128, MT], f32)
            nc.gpsimd.dma_start(msq_t[:], msq_d[:])
            acc_t = acc_pool.tile([128, N], bf16)

            # load order: group-0 patches + first bank chunks first so the
            # m-loop starts after ~2.5MB, not after the full 6MB
            at_t = at_pool.tile([128, KP, N], fp8)
            bt_t = bt_pool.tile([128, KP, M], fp8)

            def load_at(g):
                for ci in range(KP):
                    nc.gpsimd.dma_start(
                        at_t[:, ci, bass.ts(g, N // NG)],
                        at_d[:, ci, bass.ts(g, N // NG)],
                    )

            def load_bt(j):
                for ci in range(KP):
                    nc.gpsimd.dma_start(
                        bt_t[:, ci, bass.ts(j, M // 8)],
                        bt_d[:, ci, bass.ts(j, M // 8)],
                    )

            load_at(0)
            load_bt(0)
            load_bt(1)
            for g in range(1, NG):
                load_at(g)
                load_bt(2 * g)
                load_bt(2 * g + 1)

            def fill_quad(ps, m, nt0, mmw=512):
                for h in range(2048 // mmw):
                    for c in range(2):
                        nc.tensor.matmul(
                            ps[:, h * mmw : (h + 1) * mmw],
                            bt_t[:, 2 * c : 2 * c + 2, bass.ts(m, 128)],
                            at_t[:, 2 * c : 2 * c + 2,
                                 nt0 * 512 + h * mmw :
                                 nt0 * 512 + (h + 1) * mmw],
                            start=(c == 0),
                            stop=(c == 1),
                            perf_mode=mybir.MatmulPerfMode.DoubleRow,
                        )

            def compute_quads():
                # Per m-step: two [128,2048] quads (4+4 banks). 3 of every
                # 4 quads drain via ACT evac + DVE merge; q1 on odd m via
                # one DVE scalar_tensor_tensor straight from PSUM, so ACT
                # and DVE split the PSUM-drain load ~3:1.
                for g in range(NG):
                    for m in range(MT):
                        for q in range(GT // 4):
                            nt0 = g * GT + 4 * q
                            a = acc_t[:, nt0 * 512 : (nt0 + 4) * 512]
                            ps = psum_pool.tile([128, 2048], f32)
                            fill_quad(ps, m, nt0, mmw)
                            if m == 0:
                                nc.scalar.activation(
                                    a, ps[:],
                                    mybir.ActivationFunctionType.Identity,
                                    bias=msq_t[:, m : m + 1], scale=2.0,
                                )
                            elif q == 1 and m % 2 == 1:
                                nc.vector.scalar_tensor_tensor(
                                    a, ps[:], msq_t[:, m : m + 1], a,
                                    mybir.AluOpType.add, mybir.AluOpType.max,
                                )
                            else:
                                ev = ev_pool.tile([128, 2048], bf16)
                                nc.scalar.activation(
                                    ev[:], ps[:],
                                    mybir.ActivationFunctionType.Identity,
                                    bias=msq_t[:, m : m + 1], scale=2.0,
                                )
                                nc.vector.tensor_max(a, a, ev[:])
                        if m == MT - 1:
                            for j in range(GT):
                                col = (g * GT + j) * 512
                                nc.gpsimd.dma_start(
                                    out_d[:, col : col + 512],
                                    acc_t[:, col : col + 512],
                                )

            compute_body = compute_quads

            if repeat == 1:
                compute_body()
            else:
                with tc.For_i(0, repeat):
                    compute_body()

    nc.finalize()
    return nc


_NC = None


def prepare_in_maps(patches: np.ndarray, memory_bank: np.ndarray):
    m_sq = np.sum(memory_bank.astype(np.float64) ** 2, axis=1)
    C = float(np.mean(m_sq))
    at_np = np.ascontiguousarray(
        patches.T.astype(ml_dtypes.float8_e4m3)
        .reshape(KP, 128, N).transpose(1, 0, 2)
    )
    in_maps = []
    for c in range(N_CORES):
        bank_c = memory_bank[c * M : (c + 1) * M]
        bt_np = np.ascontiguousarray(
            bank_c.T.astype(ml_dtypes.float8_e4m3)
            .reshape(KP, 128, M).transpose(1, 0, 2)
        )
        msq_c = C - m_sq[c * M : (c + 1) * M]
        msq_np = np.ascontiguousarray(
            msq_c.reshape(MT, 128).T
        ).astype(np.float32)
        in_maps.append({"at": at_np, "bt": bt_np, "msq": msq_np})
    return in_maps


def kernel(patches: np.ndarray, memory_bank: np.ndarray) -> np.ndarray:
    global _NC
    if _NC is None:
        _NC = _build_nc()
    nc = _NC

    p64 = patches.astype(np.float64)
    b64 = memory_bank.astype(np.float64)
    p_sq = np.sum(p64 * p64, axis=1)          # [N]
    m_sq = np.sum(b64 * b64, axis=1)          # [M_TOTAL]
    C = float(np.mean(m_sq))

    in_maps = prepare_in_maps(patches, memory_bank)

    br = run_bass_kernel_spmd(nc, in_maps, list(range(N_CORES)))
    r = np.max(
        np.stack(
            [np.asarray(br.results[c]["out"], np.float64).max(axis=0)
             for c in range(N_CORES)]
        ),
        axis=0,
    )
    min_d2 = np.maximum(p_sq + C - r, 0.0)

    # Host refinement: device min_d2 is approximate (fp8 matmul + bf16 max
    # accumulation). Recompute exact d2 rows for every candidate patch whose
    # approx score is within EPS of the max. Correctness needs
    # EPS >= 2*max|err|; measured err is +-7 (fp8e4), so 30 is ~2x margin.
    EPS = 30.0
    amax = float(min_d2.max())
    S = np.flatnonzero(min_d2 >= amax - EPS)
    if len(S) > 2048:
        S = np.argsort(min_d2)[-2048:]
    cross_S = p64[S] @ b64.T
    d2_S = p_sq[S, None] + m_sq[None, :] - 2.0 * cross_S
    score = np.sqrt(max(float(np.maximum(d2_S, 0.0).min(axis=1).max()), 0.0))
    return np.asarray(score, dtype=np.float32)
